# revision 46
# baseline (speedup 1.0000x reference)
"""Trainium2 Bass kernel for nn_BiochemicalDiffusion.

Computes  out = F - B*x - r * rowsum(x * (A @ x))  for A:[10000,10000] f32,
x:[10000,64] f32, across 8 NeuronCores.

Sharding (all done host-side in this file):
  - A is sharded row-wise: core c gets rows [c*1250, (c+1)*1250).
  - The shard is passed pre-transposed (A_shard^T, [10000, 1250]) so the PE
    can contract over k directly: Ax_shard = A_shard^T.T @ x.
  - x is passed in full to every core (it is tiny), pre-tiled into the
    [128, 79*64] SBUF layout the matmul consumes.
  - Each core computes its [1250, 64] slice of the output; the host
    concatenates them.

Default path ("f8dr"): A is quantized to fp8e4m3 (1 byte/elem, vs 4 for
f32).  The absmax-rel error of the final output under fp8 A and x is
~1.6e-3 (quantization errors of uniform[0,1] values are zero-mean and
average out over the 10000-deep contraction), far inside the 2e-2 gate,
while cutting HBM traffic of A by 4x and matmul passes to one.  "f8dr"
additionally uses the PE DoubleRow perf mode (2 fp8 weights per cell,
contraction depth 256 per pass) to halve PE busy time, so the kernel is
purely HBM-bound (~310-330 GB/s/core sustained on the slab stream).

Hardware note: PSUM accumulation groups must not share a PSUM bank -- two
interleaved accumulation groups in one bank corrupt each other.  All
layouts below keep one live accumulation group per bank.

Everything is hardcoded to the problem shapes; kernel.py is self-contained.
"""

import numpy as np

N = 10000
DIM = 64
NCORES = 8
MSHARD = N // NCORES  # 1250 rows of A / out per core
MT = 125              # m-tile (PSUM partition) size
NMT = MSHARD // MT    # 10 m-tiles per core
KT = 128              # k-tile (contraction) size
NKT = 79              # k-tiles covering the 10000 rows (last is 16+zeros)
KPAD = NKT * KT       # 10112 (rows 10000+ are zeros; they contribute 0)

F_CONST = 1.0
B_CONST = 0.1
R_CONST = 0.01

# m-chunks: each chunk's accumulator owns one PSUM bank (<=512 f32 wide);
# widths EVEN (fp32r ISA restriction, harmless for the other dtypes)
MCH = [(0, 418), (418, 834), (834, 1250)]

# --- legacy (split/splitf8/bf16/f32r) DMA grouping: 4 k-tiles/transfer ---
KQ = 4
KGROUPS = ([(0, 1), (1, 1), (2, 2)]
           + [(k0, 4) for k0 in range(4, 76, 4)]
           + [(76, 3)])
NG = len(KGROUPS)                     # 22 groups covering 79 tiles

# --- fp8 path DMA grouping: up to 8 k-tiles (~1.28 MB) per transfer.
# First groups small so the first matmul starts early (pipeline ramp).
KQ8 = 8
KG_F8 = ([(0, 1), (1, 1), (2, 2), (4, 4)]
         + [(k0, 8) for k0 in range(8, 72, 8)]
         + [(72, 7)])
# DoubleRow needs k-tile PAIRS inside one slab tile -> even-aligned groups.
# k-tile 78 (16 real rows) rides alone and runs as a plain fp8 matmul --
# placed FIRST: summation order is irrelevant, its 20 KB transfer doubles
# as pipeline ramp, and the tail then ends on the tapered DR pairs.
# The END is tapered (4, 2 tiles) so the final group's matmuls -- which
# can only start after its whole transfer lands -- add <1us of tail.
KG_F8DR = ([(78, 1), (0, 2), (2, 2), (4, 4)]
           + [(k0, 8) for k0 in range(8, 72, 8)]
           + [(72, 4), (76, 2)])
# 6-tile steady-state groups: PE idle per group ~1.6us (vs 2.1), keeping
# the PE inside the HAM busy window throughout the stream
KG6_F8DR = ([(78, 1), (0, 2), (2, 2), (4, 4)]
            + [(k0, 6) for k0 in range(8, 74, 6)]
            + [(74, 2), (76, 2)])
# smoothed ramp: two extra 4-tile groups so the PE is never starved >2.1us
# during pipe-fill (avoids the early HAM re-throttle)
KGR_F8DR = ([(78, 1), (0, 2), (2, 2), (4, 4), (8, 4), (12, 4)]
            + [(k0, 8) for k0 in range(16, 72, 8)]
            + [(72, 4), (76, 2)])
# no ramp: straight to 8-tile groups.  Bucketed traces show the stream
# sustains ~425 GB/s (fabric ceiling) on 8-tile groups but only ~170-300
# during the small-group ramp; PE start time is irrelevant (2x slack), so
# big groups from the first byte shorten the stream by ~3us.
KGB_F8DR = ([(78, 1)]
            + [(k0, 8) for k0 in range(0, 72, 8)]
            + [(72, 4), (76, 2)])
KGE_F8DR = ([(78, 1), (0, 4)]
            + [(k0, 8) for k0 in range(4, 68, 8)]
            + [(68, 4), (72, 4), (76, 2)])
# f8drc: KGB groups + x/consts on the scalar HWDGE ring (idle until the
# tail) instead of SWDGE -- HWDGE moves the same bytes with far fewer
# descriptor stalls during the early window

# variant with ~2.6 MB steady-state transfers
# (DO NOT USE: wedges the device -- NRT_EXEC_UNIT_UNRECOVERABLE)
KG16_F8DR = ([(0, 2), (2, 2), (4, 4), (8, 8), (16, 16), (32, 16), (48, 16),
              (64, 14), (78, 1)])
# stationary x pieces (slices of the NKT axis): piece 1 covers the first
# groups so matmuls start ~10us in; the rest streams behind on the same
# SWDGE queue.  Tile 78 comes first to match the group order.
XT_PIECES = [(NKT - 1, NKT), (0, 8), (8, 24), (24, NKT - 1)]
# per-k-tile slab pitch: DoubleRow's 3D weight/ifmap APs want 16B-aligned
# k-sub strides; 1264 = 79*16 (14 zero pad cols per k-tile, 1.1% extra DMA)
AWID_DR = 1264
KLAST = 16            # real k-rows in the final k-tile (10000 - 78*128)

A_LO_SCALE = 512.0  # fp8 A_lo is stored pre-scaled into [-1, 1] (splitf8)

DEFAULT_LAYOUT = "x_stat"
# f8drd = fp8 DoubleRow, ramped slab groups, x/epilogue consts on the
# scalar HWDGE ring (beats SWDGE consts by ~1.5us in both A/B orders:
# fewer SDMA descriptor stalls during the early window, less PE throttle)
DEFAULT_MM_DTYPE = "f8drd"

_nc_cache = {}


def _dtypes(mm_dtype):
    from concourse import mybir
    mm = {
        "f32": mybir.dt.float32,
        "f32r": mybir.dt.float32r,
        "bf16": mybir.dt.bfloat16,
        "split": mybir.dt.bfloat16,
        "splitf8": mybir.dt.bfloat16,
        "f8": mybir.dt.float8e4,
        "f8dr": mybir.dt.float8e4,
        "f8dr16": mybir.dt.float8e4,
        "f8drn": mybir.dt.float8e4,
        "f8dr6": mybir.dt.float8e4,
        "f8drr": mybir.dt.float8e4,
        "f8drb": mybir.dt.float8e4,
        "f8drc": mybir.dt.float8e4,
        "f8drd": mybir.dt.float8e4,
        "f8dre": mybir.dt.float8e4,
    }[mm_dtype]
    return mm, mybir.dt.float32


def _f8_cfg(mm_dtype):
    """(kgroups, awid, dr) for the fp8 variants."""
    if mm_dtype == "f8":
        return KG_F8, MSHARD, False
    if mm_dtype == "f8dr":
        return KG_F8DR, AWID_DR, True
    if mm_dtype == "f8dr16":
        return KG16_F8DR, AWID_DR, True
    if mm_dtype == "f8drn":
        # unpadded k-sub stride (1250 B, not 16B-aligned) -- experimental
        return KG_F8DR, MSHARD, True
    if mm_dtype == "f8dr6":
        return KG6_F8DR, AWID_DR, True
    if mm_dtype == "f8drr":
        return KGR_F8DR, AWID_DR, True
    if mm_dtype == "f8drb":
        return KGB_F8DR, AWID_DR, True
    if mm_dtype == "f8drc":
        return KGB_F8DR, AWID_DR, True
    if mm_dtype == "f8drd":
        # champion ramp, but consts on the scalar HWDGE ring (no SWDGE)
        return KG_F8DR, AWID_DR, True
    if mm_dtype == "f8dre":
        # f8drd + faster ramp (4-tile then 8s): bigger early transfers run
        # closer to the fabric ceiling; inter-group gaps stay < HAM window
        return KGE_F8DR, AWID_DR, True
    raise ValueError(mm_dtype)


def _np_mm_dtype(mm_dtype):
    if mm_dtype in ("bf16", "split", "splitf8"):
        import ml_dtypes
        return np.dtype(ml_dtypes.bfloat16)
    if mm_dtype in ("f8", "f8dr", "f8dr16", "f8drn", "f8dr6", "f8drr", "f8drb", "f8drc", "f8drd", "f8dre"):
        import ml_dtypes
        return np.dtype(ml_dtypes.float8_e4m3)
    return np.dtype(np.float32)


def _body_f8(ctx, tc, a_t, xt_d, xstr_d, x65_d, waff_d, out_d, mm_dtype):
    """Pure-fp8 path.  x k-tiles are the stationary operand, A^T slabs
    stream as the moving operand.  One matmul pass (fp8 hi/lo split not
    needed for the 2e-2 gate).  dr=True uses DoubleRow: each matmul
    contracts a PAIR of k-tiles (256 deep) at ~1 output column/cycle.

    DMA: A^T streams in up to 8-k-tile slabs (~1.28 MB/transfer, host
    pre-tiled so each group is ONE flat 2D DMA) on the sync HWDGE ring,
    which stays dedicated to the slab stream; x and the epilogue
    constants ride the gpsimd (SWDGE) queue, which adds bandwidth ON TOP
    of the HWDGE stream instead of delaying it.

    Epilogue (all in the natural [64, 1250] layout; host un-transposes):
      bc   = waff^T @ [x^T; 1]           (PE, mid-stream: F - B*x^T)
      p    = (-r * x^T) .* Ax^T          (DVE, bf16 out)     per chunk
      bc  += J_ones64^T @ p              (PE: column sums of p broadcast
                                          to all 64 rows, accumulated)
      o    = bc                          (ACT copy, PSUM -> SBUF)
    then one DMA per chunk.  No per-m-tile transpose chains, and the DVE
    critical path is just the three multiplies."""
    import concourse.bass  # noqa: F401
    from concourse import mybir

    nc = tc.nc
    f32 = mybir.dt.float32
    bf16 = mybir.dt.bfloat16
    fp8 = mybir.dt.float8e4

    kgroups, awid, dr = _f8_cfg(mm_dtype)
    kq = max(g for _, g in kgroups)
    nbufs = 5 if kq <= 8 else 4

    consts = ctx.enter_context(tc.tile_pool(name="consts", bufs=1))
    slabs = ctx.enter_context(tc.tile_pool(name="slabs", bufs=nbufs))
    psums = ctx.enter_context(tc.tile_pool(name="psums", bufs=1, space="PSUM"))
    bcp = ctx.enter_context(tc.tile_pool(name="bcp", bufs=1, space="PSUM"))
    epil = ctx.enter_context(tc.tile_pool(name="epil", bufs=1))

    # stationary x tiles: SWDGE (gpsimd) queue so they ride ALONGSIDE the
    # HWDGE slab stream (SWDGE adds bandwidth on top of the sync ring
    # rather than delaying it); split into pieces so the first matmul only
    # waits for the first ~33 KB piece.
    cq = nc.scalar if mm_dtype in ("f8drc", "f8drd", "f8dre") else nc.gpsimd
    xt = consts.tile([KT, NKT, DIM], fp8)
    for z0, z1 in XT_PIECES:
        cq.dma_start(out=xt[:, z0:z1, :], in_=xt_d[:, z0:z1, :])
    # epilogue constants (needed only ~40us in; const ring is idle then)
    xstr = consts.tile([DIM, MSHARD], bf16)
    cq.dma_start(out=xstr, in_=xstr_d)
    x65 = consts.tile([DIM + 1, MSHARD], bf16)
    cq.dma_start(out=x65, in_=x65_d)
    waff = consts.tile([DIM + 1, DIM], bf16)
    cq.dma_start(out=waff, in_=waff_d)
    jones = consts.tile([DIM, DIM], bf16)
    nc.vector.memset(jones, 1.0)

    accs = [psums.tile([DIM, c1 - c0], f32, name=f"acc{i}", tag=f"acc{i}")
            for i, (c0, c1) in enumerate(MCH)]
    bcs = [bcp.tile([DIM, c1 - c0], f32, name=f"bc{i}", tag=f"bc{i}")
           for i, (c0, c1) in enumerate(MCH)]

    affine_issued = False
    for gi, (k0, g) in enumerate(kgroups):
        if not affine_issued and 48 <= k0 < NKT - 1:
            # affine part of the epilogue: bc = waff^T @ [x^T; 1] =
            # F - B*x^T.  Independent of the accumulators, so it runs in
            # the PE's idle slots mid-stream instead of on the tail.
            affine_issued = True
            for i, (c0, c1) in enumerate(MCH):
                nc.tensor.matmul(bcs[i], lhsT=waff, rhs=x65[:, c0:c1],
                                 start=True, stop=False)
        slab = slabs.tile([KT, kq, awid], fp8, name=f"slab{gi}", tag="slab")
        dma_eng = nc.sync
        padded_tile = dr and k0 + g == NKT and g == 1
        if padded_tile:
            # k-tile 78 has only 16 real rows (9984..9999); skip the pad
            dma_eng.dma_start(out=slab[:KLAST, :g, :],
                              in_=a_t[gi * KT:gi * KT + KLAST, :g, :])
        else:
            dma_eng.dma_start(out=slab[:, :g, :],
                              in_=a_t[gi * KT:(gi + 1) * KT, :g, :])
        if dr and g > 1:
            for s in range(0, g, 2):
                kt = k0 + s
                for i, (c0, c1) in enumerate(MCH):
                    nc.tensor.matmul(
                        accs[i],
                        lhsT=xt[:, kt:kt + 2, :],
                        rhs=slab[:, s:s + 2, c0:c1],
                        start=False,     # group opened by k-tile 78 (first)
                        stop=(kt == NKT - 3),
                        perf_mode=mybir.MatmulPerfMode.DoubleRow,
                    )
        else:
            for sub in range(g):
                kt = k0 + sub
                kp = KLAST if (dr and kt == NKT - 1) else KT
                for i, (c0, c1) in enumerate(MCH):
                    nc.tensor.matmul(
                        accs[i],
                        lhsT=xt[:kp, kt:kt + 1, :],
                        rhs=slab[:kp, sub:sub + 1, c0:c1],
                        start=(kt == (NKT - 1 if dr else 0)),
                        stop=(False if dr else kt == NKT - 1),
                    )

    for i, (c0, c1) in enumerate(MCH):
        w = c1 - c0
        p = epil.tile([DIM, w], bf16, name=f"p{i}", tag=f"p{i}")
        nc.vector.tensor_mul(p, xstr[:, c0:c1], accs[i])
        # bc += J^T @ p  ->  bc = (F - B*x^T) - r * colsum(x^T .* Ax^T)
        nc.tensor.matmul(bcs[i], lhsT=jones, rhs=p, start=False, stop=True)
        o = epil.tile([DIM, w], f32, name=f"o{i}", tag=f"o{i}")
        # PSUM -> SBUF move on the ACT engine; DVE only does the mults
        nc.scalar.copy(o, bcs[i])
        # sync ring is idle once the slab stream ends; spread the output
        # chunks across both HWDGE rings
        dma_eng = nc.scalar if i % 2 == 0 else nc.sync
        dma_eng.dma_start(out=out_d[:, c0:c1], in_=o)


def _body_x_stat(ctx, tc, a_t, a_l, xt_d, xt8_d, xs_d, xst_d, id_d, out_d,
                 mmdt, mm_dtype):
    """k-outer loop; x k-tiles are the stationary operand, A^T slabs stream
    as the moving operand (large free dim -> full-rate fp32r / bf16).
    Produces Ax^T in PSUM (3 chunk accumulators, one bank each); epilogue
    transposes x^T*Ax^T back via the PE.

    DMA streams in KQ-k-tile groups (~1.3-2.6 MB per transfer) to amortize
    per-DMA overhead; the stationary x is preloaded in per-group chunks on
    the gpsimd queue so the first matmul does not wait for the whole x.

    split: A and x decomposed as hi+lo bf16 pairs; A@x ~= A_hi@x_hi +
    A_lo@x_hi + A_hi@x_lo.  a_t holds [A_hi^T | A_lo^T] side by side; xt
    holds [x_hi | x_lo] per k-tile so the two x terms ride in ONE 128-wide
    stationary: pass A computes both x_hi@A_hi (psum rows 0:64) and
    x_lo@A_hi (rows 64:128) in a single moving sweep of the A_hi slab
    half; pass B computes x_hi@A_lo.

    splitf8: like split but A_lo is a SEPARATE fp8e4m3 tensor pre-scaled
    by A_LO_SCALE, and pass B runs all-fp8 (x in fp8) -- 3 bytes/element
    of A traffic instead of 4; epilogue rescales the pass-B accumulator."""
    import concourse.bass  # noqa: F401
    from concourse import mybir

    nc = tc.nc
    f32 = mybir.dt.float32
    fp8 = mybir.dt.float8e4
    split = mm_dtype in ("split", "splitf8")
    f8 = mm_dtype == "splitf8"

    consts = ctx.enter_context(tc.tile_pool(name="consts", bufs=1))
    slabs = ctx.enter_context(tc.tile_pool(name="slabs", bufs=6))
    psums = ctx.enter_context(tc.tile_pool(name="psums", bufs=1, space="PSUM"))
    ptp = ctx.enter_context(tc.tile_pool(name="ptp", bufs=2, space="PSUM"))
    epil = ctx.enter_context(tc.tile_pool(name="epil", bufs=2))

    # elements per k-row in the a_t tensor.  For splitf8 the hi (bf16) and
    # lo (fp8) halves are byte-packed into one bf16-typed stream:
    # per k-tile per partition = 1250 bf16 hi elems then 1250 fp8 lo bytes
    # (= 625 bf16-elem slots); pass B reads the lo region via bitcast.
    awid = 2 * MSHARD if (split and not f8) else MSHARD
    if f8:
        awid = MSHARD + MSHARD // 2  # 1875 bf16 elems per k-tile
    xwid = 2 * DIM if split else DIM  # stationary block width per k-tile

    xt = consts.tile([KT, NKT * xwid], mmdt)
    if f8:
        xt8 = consts.tile([KT, NKT * DIM], fp8)
    bcol = consts.tile([MT, 1], f32)
    nc.vector.memset(bcol, -B_CONST)

    accs = [psums.tile([xwid, c1 - c0], f32, name=f"acc{i}", tag=f"acc{i}")
            for i, (c0, c1) in enumerate(MCH)]
    if split:
        accs_lo = [psums.tile([DIM, c1 - c0], f32, name=f"accl{i}",
                              tag=f"accl{i}")
                   for i, (c0, c1) in enumerate(MCH)]

    for gi, (k0, g) in enumerate(KGROUPS):
        # stationary chunk for this group's k-tiles (gpsimd queue, overlaps
        # with the slab stream on the sync queue)
        nc.gpsimd.dma_start(
            out=xt[:, k0 * xwid:(k0 + g) * xwid],
            in_=xt_d[:, k0 * xwid:(k0 + g) * xwid],
        )
        if f8:
            nc.gpsimd.dma_start(
                out=xt8[:, k0 * DIM:(k0 + g) * DIM],
                in_=xt8_d[:, k0 * DIM:(k0 + g) * DIM],
            )
        slab = slabs.tile([KT, KQ * awid], mmdt, name=f"slab{gi}", tag="slab")
        nc.sync.dma_start(out=slab[:, :g * awid],
                          in_=a_t[gi * KT:(gi + 1) * KT, :g * awid])

        for sub in range(g):
            kt = k0 + sub
            xoff = kt * xwid
            base = sub * MSHARD if f8 else sub * awid
            for i, (c0, c1) in enumerate(MCH):
                # pass A: [x_hi | x_lo] (or plain x) against the A_hi half
                nc.tensor.matmul(
                    accs[i],
                    lhsT=xt[:, xoff:xoff + xwid],
                    rhs=slab[:, base + c0:base + c1],
                    start=(kt == 0),
                    stop=(kt == NKT - 1),
                )
            if split:
                for i, (c0, c1) in enumerate(MCH):
                    # pass B: x_hi (bf16) or x (fp8) against the A_lo half
                    if f8:
                        off = g * MSHARD + (sub * MSHARD + c0) // 2
                        rhs = slab[:, off:off + (c1 - c0) // 2].bitcast(fp8)
                        lo_lhs = xt8[:, kt * DIM:(kt + 1) * DIM]
                    else:
                        rhs = slab[:, base + MSHARD + c0:base + MSHARD + c1]
                        lo_lhs = xt[:, xoff:xoff + DIM]
                    nc.tensor.matmul(
                        accs_lo[i],
                        lhsT=lo_lhs,
                        rhs=rhs,
                        start=(kt == 0),
                        stop=(kt == NKT - 1),
                    )

    # epilogue-only constants: issued after the slab stream in program
    # order so they don't delay the first matmuls; they transfer during
    # the main loop and are ready long before the epilogue needs them.
    xs = consts.tile([MT, NMT * DIM], f32)
    nc.gpsimd.dma_start(out=xs, in_=xs_d)
    xst = consts.tile([DIM, MSHARD], f32)
    nc.gpsimd.dma_start(out=xst, in_=xst_d)
    ident = consts.tile([DIM, DIM], f32)
    nc.gpsimd.dma_start(out=ident, in_=id_d)

    # P = x^T * Ax^T  (elementwise), [64, 1250] in SBUF
    p_full = epil.tile([DIM, MSHARD], f32, bufs=1)
    for i, (c0, c1) in enumerate(MCH):
        w = c1 - c0
        if split:
            # only one PSUM operand allowed per DVE op -> chain via SBUF
            tsum = epil.tile([DIM, w], f32, name=f"tsum{i}", tag="tsum")
            nc.vector.tensor_copy(tsum, accs[i][0:DIM, :])
            nc.vector.tensor_add(tsum, tsum, accs[i][DIM:2 * DIM, :])
            if f8:
                tlo = epil.tile([DIM, w], f32, name=f"tlo{i}", tag="tlo")
                nc.vector.tensor_scalar(
                    out=tlo, in0=accs_lo[i], scalar1=1.0 / A_LO_SCALE,
                    scalar2=None, op0=mybir.AluOpType.mult)
                nc.vector.tensor_add(tsum, tsum, tlo)
            else:
                nc.vector.tensor_add(tsum, tsum, accs_lo[i])
            nc.vector.tensor_mul(p_full[:, c0:c1], xst[:, c0:c1], tsum)
        else:
            nc.vector.tensor_mul(p_full[:, c0:c1], xst[:, c0:c1], accs[i])

    for mt in range(NMT):
        pt = ptp.tile([MT, DIM], f32, name=f"pt{mt}", tag="pt")
        nc.tensor.transpose(
            out=pt, in_=p_full[:, mt * MT:(mt + 1) * MT], identity=ident,
        )
        s = epil.tile([MT, 1], f32, name=f"s{mt}", tag="s")
        nc.vector.tensor_reduce(
            out=s, in_=pt, axis=mybir.AxisListType.X, op=mybir.AluOpType.add,
        )
        t_col = epil.tile([MT, 1], f32, name=f"t{mt}", tag="t")
        # t = s * (-r) + F
        nc.vector.tensor_scalar(
            out=t_col, in0=s, scalar1=-R_CONST, scalar2=F_CONST,
            op0=mybir.AluOpType.mult, op1=mybir.AluOpType.add,
        )
        o = epil.tile([MT, DIM], f32, name=f"o{mt}", tag="o")
        nc.vector.tensor_scalar(
            out=o, in0=xs[:, mt * DIM:(mt + 1) * DIM], scalar1=bcol,
            scalar2=t_col, op0=mybir.AluOpType.mult, op1=mybir.AluOpType.add,
        )
        nc.sync.dma_start(out=out_d[mt * MT:(mt + 1) * MT, :], in_=o)


def build(layout=None, mm_dtype=None):
    layout = layout or DEFAULT_LAYOUT
    mm_dtype = mm_dtype or DEFAULT_MM_DTYPE
    key = (layout, mm_dtype)
    if key in _nc_cache:
        return _nc_cache[key]

    from contextlib import ExitStack
    import concourse.tile as tile
    from concourse import bacc

    mmdt, f32 = _dtypes(mm_dtype)

    nc = bacc.Bacc(
        "TRN2",
        target_bir_lowering=False,
        debug=False,
        enable_asserts=False,
        num_devices=NCORES,
        name=f"biochem_{layout}_{mm_dtype}",
    )
    from concourse import mybir

    if mm_dtype in ("f8", "f8dr", "f8dr16", "f8drn", "f8dr6", "f8drr", "f8drb", "f8drc", "f8drd", "f8dre"):
        kgroups, awid, dr = _f8_cfg(mm_dtype)
        kq = max(g for _, g in kgroups)
        ng = len(kgroups)
        a_t = nc.dram_tensor(
            "a_t", [ng * KT, kq, awid], mmdt, kind="ExternalInput").ap()
        xt_d = nc.dram_tensor(
            "xt", [KT, NKT, DIM], mmdt, kind="ExternalInput").ap()
        xstr_d = nc.dram_tensor(
            "xstr", [DIM, MSHARD], mybir.dt.bfloat16, kind="ExternalInput").ap()
        x65_d = nc.dram_tensor(
            "x65", [DIM + 1, MSHARD], mybir.dt.bfloat16,
            kind="ExternalInput").ap()
        waff_d = nc.dram_tensor(
            "waff", [DIM + 1, DIM], mybir.dt.bfloat16,
            kind="ExternalInput").ap()
        out_d = nc.dram_tensor(
            "out", [DIM, MSHARD], f32, kind="ExternalOutput").ap()
        with tile.TileContext(nc) as tc:
            with ExitStack() as ctx:
                _body_f8(ctx, tc, a_t, xt_d, xstr_d, x65_d, waff_d, out_d,
                         mm_dtype)
        nc.compile()
        _nc_cache[key] = nc
        return nc

    split = mm_dtype in ("split", "splitf8")
    f8 = mm_dtype == "splitf8"
    awid = 2 * MSHARD if (split and not f8) else MSHARD
    if f8:
        awid = MSHARD + MSHARD // 2  # byte-packed hi(bf16)+lo(fp8)
    xwid = 2 * DIM if split else DIM
    # a_t is pre-tiled host-side into slab layout: row gi*128+p holds the
    # p-th partition of DMA group gi ([KQ consecutive k-rows] worth of data)
    a_t = nc.dram_tensor(
        "a_t", [NG * KT, KQ * awid], mmdt, kind="ExternalInput").ap()
    a_l = xt8_d = None
    if f8:
        xt8_d = nc.dram_tensor(
            "xt8", [KT, NKT * DIM], mybir.dt.float8e4,
            kind="ExternalInput").ap()
    xt_d = nc.dram_tensor("xt", [KT, NKT * xwid], mmdt, kind="ExternalInput").ap()
    xs_d = nc.dram_tensor("xs", [MT, NMT * DIM], f32, kind="ExternalInput").ap()
    if layout == "x_stat":
        xst_d = nc.dram_tensor("xst", [DIM, MSHARD], f32, kind="ExternalInput").ap()
        id_d = nc.dram_tensor("ident", [DIM, DIM], f32, kind="ExternalInput").ap()
    out_d = nc.dram_tensor("out", [MSHARD, DIM], f32, kind="ExternalOutput").ap()

    with tile.TileContext(nc) as tc:
        with ExitStack() as ctx:
            if layout == "x_stat":
                _body_x_stat(ctx, tc, a_t, a_l, xt_d, xt8_d, xs_d, xst_d,
                             id_d, out_d, mmdt, mm_dtype)
            else:
                raise ValueError(layout)
    nc.compile()
    _nc_cache[key] = nc
    return nc


def prepare_in_maps(x, A, layout=None, mm_dtype=None):
    layout = layout or DEFAULT_LAYOUT
    mm_dtype = mm_dtype or DEFAULT_MM_DTYPE
    np_mm = _np_mm_dtype(mm_dtype)

    x = np.asarray(x, np.float32)
    A = np.asarray(A, np.float32)

    if mm_dtype in ("f8", "f8dr", "f8dr16", "f8drn", "f8dr6", "f8drr", "f8drb", "f8drc", "f8drd", "f8dre"):
        kgroups, awid, dr = _f8_cfg(mm_dtype)
        kq = max(g for _, g in kgroups)
        ng = len(kgroups)

        xp = np.zeros((KPAD, DIM), np_mm)
        xp[:N] = x.astype(np_mm)
        # [KPAD, DIM] -> [KT, NKT, DIM]
        xt_np = np.ascontiguousarray(
            xp.reshape(NKT, KT, DIM).transpose(1, 0, 2))

        in_maps = []
        for c in range(NCORES):
            sh = slice(c * MSHARD, (c + 1) * MSHARD)
            at = np.zeros((KPAD, awid), np_mm)
            at[:N, :MSHARD] = np.ascontiguousarray(A[sh].T).astype(np_mm)
            # slab pre-tiling: a_t[gi*128+p, sub, :] = at[(k0+sub)*128+p, :]
            a_t_c = np.zeros((ng * KT, kq, awid), np_mm)
            for gi, (k0, g) in enumerate(kgroups):
                blk = at[k0 * KT:(k0 + g) * KT, :]
                a_t_c[gi * KT:(gi + 1) * KT, :g, :] = (
                    blk.reshape(g, KT, awid).transpose(1, 0, 2)
                )
            import ml_dtypes
            bf = np.dtype(ml_dtypes.bfloat16)
            xsh_t = np.ascontiguousarray(x[sh].T)          # [64, 1250]
            x65_np = np.ones((DIM + 1, MSHARD), np.float32)
            x65_np[:DIM] = xsh_t
            waff_np = np.zeros((DIM + 1, DIM), np.float32)
            waff_np[:DIM, :DIM] = -B_CONST * np.eye(DIM, dtype=np.float32)
            waff_np[DIM, :] = F_CONST
            in_maps.append({
                "a_t": a_t_c, "xt": xt_np,
                "xstr": (-R_CONST * xsh_t).astype(bf),
                "x65": x65_np.astype(bf),
                "waff": waff_np.astype(bf),
            })
        return in_maps

    split = mm_dtype in ("split", "splitf8")
    f8 = mm_dtype == "splitf8"
    if f8:
        import ml_dtypes
        np_fp8 = np.dtype(ml_dtypes.float8_e4m3)

    def tile_k(arr):
        """[KPAD, W] -> [KT, NKT*W] SBUF layout, padded rows are zero."""
        w = arr.shape[1]
        xp = np.zeros((KPAD, w), arr.dtype)
        xp[:N] = arr
        return np.ascontiguousarray(
            xp.reshape(NKT, KT, w).transpose(1, 0, 2).reshape(KT, NKT * w)
        )

    xt8_np = None
    if split:
        x_hi = x.astype(np_mm)
        x_lo = (x - x_hi.astype(np.float32)).astype(np_mm)
        # per k-tile stationary block is [x_hi | x_lo], 128 wide
        xt_np = tile_k(np.concatenate([x_hi, x_lo], axis=1))
        if f8:
            xt8_np = tile_k(x.astype(np_fp8))
    else:
        xt_np = tile_k(x).astype(np_mm)

    ident = np.eye(DIM, dtype=np.float32)

    def tile_slabs(at):
        """[KPAD, W] -> [NG*128, KQ*W] host pre-tiling into slab layout:
        row gi*128+p, cols sub*W:(sub+1)*W  =  at[(k0+sub)*128 + p, :]
        for group gi=(k0, g); unused columns of small groups stay zero."""
        w = at.shape[1]
        out = np.zeros((NG * KT, KQ * w), at.dtype)
        for gi, (k0, g) in enumerate(KGROUPS):
            blk = at[k0 * KT:(k0 + g) * KT, :]
            out[gi * KT:(gi + 1) * KT, :g * w] = (
                blk.reshape(g, KT, w).transpose(1, 0, 2).reshape(KT, g * w)
            )
        return out

    def pad_k(at):
        out = np.zeros((KPAD, at.shape[1]), at.dtype)
        out[:N] = at
        return out

    in_maps = []
    for c in range(NCORES):
        sh = slice(c * MSHARD, (c + 1) * MSHARD)
        at_f32 = pad_k(np.ascontiguousarray(A[sh].T))
        if f8:
            a_hi = at_f32.astype(np_mm)
            a_lo = at_f32 - a_hi.astype(np.float32)
            hi_t = tile_slabs(a_hi)                                # bf16
            lo_t = tile_slabs((a_lo * A_LO_SCALE).astype(np_fp8))  # fp8
            # byte-pack: per group row block, [g*2500 B hi][g*1250 B lo]
            awid = MSHARD + MSHARD // 2
            a_t_c = np.zeros((NG * KT, KQ * awid), np_mm)
            ob = a_t_c.view(np.uint8)
            hb = hi_t.view(np.uint8)
            lb = lo_t.view(np.uint8)
            for gi, (k0, g) in enumerate(KGROUPS):
                r = slice(gi * KT, (gi + 1) * KT)
                ob[r, :g * 2 * MSHARD] = hb[r, :g * 2 * MSHARD]
                ob[r, g * 2 * MSHARD:g * 3 * MSHARD] = lb[r, :g * MSHARD]
        elif split:
            a_hi = at_f32.astype(np_mm)
            a_lo = (at_f32 - a_hi.astype(np.float32)).astype(np_mm)
            a_t_c = tile_slabs(np.concatenate([a_hi, a_lo], axis=1))
        else:
            a_t_c = tile_slabs(at_f32.astype(np_mm))
        xs_c = np.ascontiguousarray(
            x[sh].reshape(NMT, MT, DIM).transpose(1, 0, 2).reshape(MT, NMT * DIM)
        )
        m = {"a_t": a_t_c, "xt": xt_np, "xs": xs_c}
        if f8:
            m["xt8"] = xt8_np
        if layout == "x_stat":
            m["xst"] = np.ascontiguousarray(x[sh].T)
            m["ident"] = ident
        in_maps.append(m)
    return in_maps


def run(inputs, trace=False, layout=None, mm_dtype=None, **spmd_kwargs):
    """Returns (full_output [10000, 64] f32, BassKernelResults)."""
    from concourse.bass_utils import run_bass_kernel_spmd

    nc = build(layout, mm_dtype)
    mmd = mm_dtype or DEFAULT_MM_DTYPE
    in_maps = prepare_in_maps(inputs["x"], inputs["A"], layout, mm_dtype)
    res = run_bass_kernel_spmd(
        nc, in_maps, core_ids=list(range(NCORES)), trace=trace, **spmd_kwargs
    )
    if mmd in ("f8", "f8dr", "f8dr16", "f8drn", "f8dr6", "f8drr", "f8drb", "f8drc", "f8drd", "f8dre"):
        # fp8 path emits out^T per core; un-transpose host-side
        out = np.concatenate(
            [res.results[c]["out"].T for c in range(NCORES)], axis=0)
    else:
        out = np.concatenate(
            [res.results[c]["out"] for c in range(NCORES)], axis=0)
    return out, res


def kernel(t=None, x=None, A=None):
    out, _ = run({"x": x, "A": A})
    return out
